# revision 7
# baseline (speedup 1.0000x reference)
"""Trainium2 Bass kernel v2 for the 2-layer GATv2 + dense-skip GNN.

Key change vs v1: the per-edge gather uses gpsimd.dma_gather (K=1024-index
single-packet calls, ~6ns/slot on the Q7) instead of per-slot
indirect_dma_start (~13.7ns/slot).  dma_gather requires int16 indices, so the
gather table is stored as 4-node *lines* of 256 bf16 (512 B): line id =
node>>2 (25088 < 32767 fits int16).  Each slot gathers its whole 4-node line;
the 3 wrong subrows (and pad slots) are killed in the softmax by adding -1e8
to their logits (host-precomputed mask), so no on-chip select is needed:
softmax weights of dead subrows are exactly 0 and the weighted aggregation
ignores them.

Other changes: no-max softmax (logits are O(1); exp is shift-invariant),
e = 0.6*(Rp-Rn) + 0.4*(Ap-An) computed from signed/abs reduces of the
|att|-scaled z (the 0.6 folds into the Exp's scale), aggregation runs on the
raw gathered xl (no xr correction needed), xr/xd stay SBUF-resident, x is
host-pre-transposed so layer-1 transforms skip the PE transpose, biases are
added from a replicated SBUF row via DVE instead of 1-row matmuls, and
layer-2 transforms are fused per-tile into the layer-1 edge phase.
"""
import sys
import numpy as np
import ml_dtypes

sys.path.insert(0, "/opt/trn_rl_repo")

P = 128
H = 64
NCORES = 8
KCALL = 1024          # dma_gather indices per call (single-packet max)
CPC = 8               # slot-columns per call (KCALL/P)
CH_COLS = 48          # slot-columns per slab chunk (multiple of CPC)
BIG = 1.0e8


class Cfg:
    def __init__(self, N, F_IN, NLOC, tinfo, p1, p2, b3val, ncall, sdp):
        self.N = N
        self.F_IN = F_IN
        self.NLOC = NLOC
        self.NT = NLOC // P
        self.NPAD = NCORES * NLOC
        self.tinfo = tinfo        # list of (tile, col0, d_t) per chunk
        self.p1 = p1
        self.p2 = p2
        self.b3val = b3val
        self.NCALL = ncall        # calls per layer
        self.SDP = sdp            # padded slot-columns per layer


# ---------------------------------------------------------------- host prep

def prep_graph(edge_index, N, NLOC):
    src = np.asarray(edge_index[0], dtype=np.int64)
    dst = np.asarray(edge_index[1], dtype=np.int64)
    NT = NLOC // P
    NPAD = NCORES * NLOC
    deg = np.bincount(dst, minlength=N)
    order = np.argsort(-deg, kind="stable")
    g_of_old = np.empty(N, dtype=np.int64)
    ii = np.arange(N)
    g_of_old[order] = (ii % NCORES) * NLOC + (ii // NCORES)
    deg_sorted = deg[order]
    d_t = [max(1, int(deg_sorted[NCORES * P * t])) if NCORES * P * t < N else 1
           for t in range(NT)]

    # chunks of tiles, slot-columns padded to CPC multiples per chunk
    chunks = []          # list of list of (tile, col0_in_chunk, d_t)
    cur, cols = [], 0
    for t in range(NT):
        if cols + d_t[t] > CH_COLS and cur:
            chunks.append(cur)
            cur, cols = [], 0
        cur.append((t, cols, d_t[t]))
        cols += d_t[t]
    if cur:
        chunks.append(cur)
    ch_cols = []
    for ch in chunks:
        c = sum(d for _, _, d in ch)
        ch_cols.append(-(-c // CPC) * CPC)
    SDP = sum(ch_cols)
    NCALL = SDP // CPC

    dst_g = g_of_old[dst]
    es = np.argsort(dst_g, kind="stable")
    src_g_sorted = g_of_old[src[es]].astype(np.int64)
    dst_g_sorted = dst_g[es]
    node_start = np.searchsorted(dst_g_sorted, np.arange(NPAD), side="left")
    node_end = np.searchsorted(dst_g_sorted, np.arange(NPAD), side="right")

    # per-core padded slot grids: line idx (int16), pad mask, subrow select
    lidx = np.zeros((NCORES, P, SDP), dtype=np.int16)
    mneg = np.full((NCORES, P, SDP), -BIG, dtype=np.float32)
    m01 = np.zeros((NCORES, P, SDP, 4), dtype=ml_dtypes.bfloat16)
    for c in range(NCORES):
        base = c * NLOC
        ns = node_start[base:base + NLOC]
        ne = node_end[base:base + NLOC]
        s0 = 0
        for ch in chunks:
            for t, c0, d in ch:
                for p in range(P):
                    lp = t * P + p
                    k = min(ne[lp] - ns[lp], d)
                    if k <= 0:
                        continue
                    srcs = src_g_sorted[ns[lp]:ns[lp] + k]
                    lidx[c, p, s0 + c0:s0 + c0 + k] = (srcs >> 2).astype(np.int16)
                    mneg[c, p, s0 + c0:s0 + c0 + k] = 0.0
                    m01[c, p, np.arange(s0 + c0, s0 + c0 + k), srcs & 3] = 1.0
            s0 += -(-sum(d for _, _, d in ch) // CPC) * CPC
    assert s0 == SDP

    # idx stream: [128, NCALL*64] int16, wrapped in 16 partitions, replicated
    idx_all = np.zeros((NCORES, P, NCALL * (KCALL // 16)), dtype=np.int16)
    for c in range(NCORES):
        for k in range(NCALL):
            blk = lidx[c, :, k * CPC:(k + 1) * CPC]       # [128, 8]
            fl = blk.T.reshape(-1)                        # pos i = s*128+p
            wrapped = fl.reshape(KCALL // 16, 16).T       # [16, 64]
            for g in range(8):
                idx_all[c, g * 16:(g + 1) * 16,
                        k * 64:(k + 1) * 64] = wrapped

    # chunk info: list over chunks of (padded_cols, [(tile, col0, d)])
    tinfo = []
    for ch, pc in zip(chunks, ch_cols):
        tinfo.append((pc, ch))
    return dict(order=order, tinfo=tinfo, SDP=SDP, NCALL=NCALL,
                idx_all=idx_all, mneg=mneg,
                m01=m01.reshape(NCORES, P, SDP * 4))


def prep_layer_weights(Wl, bl, Wr, br, att, b, Wd, bd, in_perm=None):
    Wl, bl, Wr, br, att, b, Wd, bd = [np.asarray(a, np.float64) for a in
                                      (Wl, bl, Wr, br, att, b, Wd, bd)]
    if in_perm is not None:
        Wl, Wr, Wd = Wl[in_perm], Wr[in_perm], Wd[in_perm]
    s = np.where(att >= 0, 1.0, -1.0)
    a = np.maximum(np.abs(att), 1e-12)
    perm = np.argsort(-s, kind="stable")
    p_cnt = int((s > 0).sum())
    ap = a[perm]
    # cols: [Wl*|att|]perm | [Wr*|att|]perm | Wd perm   (192 wide, no bias)
    Wt = np.concatenate([
        (Wl * a[None, :])[:, perm],
        (Wr * a[None, :])[:, perm],
        Wd[:, perm]], axis=1)
    brow = np.concatenate([
        np.zeros(H),
        ((bl + br) * a)[perm],
        (bd + b + bl)[perm]])
    inva = 1.0 / ap
    return dict(Wt=Wt.astype(np.float32),
                brow=brow.astype(np.float32),
                inva=inva.astype(np.float32), sgn=s[perm].astype(np.float32),
                perm=perm, p_cnt=p_cnt)


# ------------------------------------------------------------- bass builder

def build_bass(cfg):
    import concourse.bass as bass
    import concourse.bacc as bacc
    import concourse.mybir as mybir
    import concourse.tile as tile
    from concourse.masks import make_identity
    from contextlib import ExitStack

    f32 = mybir.dt.float32
    bf16 = mybir.dt.bfloat16
    i16 = mybir.dt.int16
    X = mybir.AxisListType.X
    ADD = mybir.AluOpType.add
    SUB = mybir.AluOpType.subtract
    MULT = mybir.AluOpType.mult
    AF = mybir.ActivationFunctionType

    NT, NLOC, NPAD, F_IN = cfg.NT, cfg.NLOC, cfg.NPAD, cfg.F_IN
    SDP, NCALL = cfg.SDP, cfg.NCALL
    TW = 3 * H                     # 192
    NLINES = NPAD // 4

    nc = bacc.Bacc("TRN2", target_bir_lowering=False, num_devices=NCORES,
                   num_swdge_queues=4)

    xT_in = nc.dram_tensor("xT_loc", [F_IN, NLOC], f32, kind="ExternalInput")
    idx_in = nc.dram_tensor("idx_all", [P, NCALL * 64], i16,
                            kind="ExternalInput")
    mneg_in = nc.dram_tensor("mneg", [P, SDP], f32, kind="ExternalInput")
    m01_in = nc.dram_tensor("m01", [P, SDP * 4], bf16, kind="ExternalInput")
    wt1_in = nc.dram_tensor("wt1", [F_IN, TW], f32, kind="ExternalInput")
    wt2_in = nc.dram_tensor("wt2", [H, TW], f32, kind="ExternalInput")
    b1_in = nc.dram_tensor("b1rep", [P, TW], f32, kind="ExternalInput")
    b2_in = nc.dram_tensor("b2rep", [P, TW], f32, kind="ExternalInput")
    inva1_in = nc.dram_tensor("inva1", [P, H], f32, kind="ExternalInput")
    inva2_in = nc.dram_tensor("inva2", [P, H], f32, kind="ExternalInput")
    sgn1_in = nc.dram_tensor("sgn1", [P, H], f32, kind="ExternalInput")
    sgn2_in = nc.dram_tensor("sgn2", [P, H], f32, kind="ExternalInput")
    w3_in = nc.dram_tensor("w3rep", [P, H], f32, kind="ExternalInput")
    out3 = nc.dram_tensor("out3", [P, NT], f32, kind="ExternalOutput")

    rg = [list(range(NCORES))]

    with ExitStack() as ctx:
        tc = ctx.enter_context(tile.TileContext(nc))
        dram = ctx.enter_context(tc.tile_pool(name="dram", bufs=1, space="DRAM"))
        xl_loc = [dram.tile([NLOC, H], bf16, name=f"xl{l}_loc") for l in (1, 2)]
        xl_full = [dram.tile([NLINES, 256], bf16, name=f"xl{l}_full",
                             addr_space="Shared") for l in (1, 2)]

        const = ctx.enter_context(tc.tile_pool(name="const", bufs=1))
        ident = const.tile([P, P], f32)
        make_identity(nc, ident[:, :])
        ones1 = const.tile([1, P], f32)
        nc.vector.memset(ones1[:, :], 1.0)
        wt_s, b_s, inva_s, sgn_s = [], [], [], []
        for l, (wt_i, b_i, iv_i, sg_i, kdim) in enumerate([
                (wt1_in, b1_in, inva1_in, sgn1_in, F_IN),
                (wt2_in, b2_in, inva2_in, sgn2_in, H)]):
            w = const.tile([kdim, TW], f32, name=f"wt{l}_s")
            nc.sync.dma_start(w[:, :], wt_i[:, :])
            b = const.tile([P, TW], f32, name=f"b{l}_s")
            nc.sync.dma_start(b[:, :], b_i[:, :])
            iv = const.tile([P, H], f32, name=f"iv{l}_s")
            nc.sync.dma_start(iv[:, :], iv_i[:, :])
            sg = const.tile([P, H], bf16, name=f"sg{l}_s")
            nc.gpsimd.dma_start(sg[:, :], sg_i[:, :])
            wt_s.append(w); b_s.append(b); inva_s.append(iv); sgn_s.append(sg)
        w3_s = const.tile([P, H], f32)
        nc.sync.dma_start(w3_s[:, :], w3_in[:, :])
        mneg_s = const.tile([P, SDP], f32)
        nc.sync.dma_start(mneg_s[:, :], mneg_in[:, :])
        m01_s = const.tile([P, SDP * 4], bf16)
        nc.sync.dma_start(m01_s[:, :], m01_in[:, :])
        # residents: xr/xd per layer (bf16), den per layer (f32)
        xr_res = [const.tile([P, NT * H], bf16, name=f"xr{l}") for l in (1, 2)]
        xd_res = [const.tile([P, NT * H], f32, name=f"xd{l}") for l in (1, 2)]
        den_res = const.tile([P, NT], f32)
        out3_s = const.tile([P, NT], f32)

        psum = ctx.enter_context(tc.tile_pool(name="psum", bufs=2, space="PSUM"))
        gp = ctx.enter_context(tc.tile_pool(name="gp", bufs=2))
        zp = ctx.enter_context(tc.tile_pool(name="zp", bufs=1))
        tp = ctx.enter_context(tc.tile_pool(name="tp", bufs=2))

        def transform_tail(t, pm, layer):
            """pm: psum [P, TW] for tile t of `layer` (0/1). Write residents +
            xl table rows (bf16) to DRAM."""
            r0 = t * H
            nc.scalar.copy(xr_res[layer][:, r0:r0 + H], pm[:, H:2 * H])
            nc.scalar.copy(xd_res[layer][:, r0:r0 + H], pm[:, 2 * H:TW])
            ot = tp.tile([P, H], bf16, tag="ot")
            nc.scalar.copy(ot[:, :], pm[:, 0:H])
            nc.sync.dma_start(xl_loc[layer][t * P:(t + 1) * P, :], ot[:, :])

        # ---- phase T1: layer-1 transforms from pre-transposed x
        for t in range(NT):
            xt = tp.tile([F_IN, P], f32, tag="xt")
            nc.sync.dma_start(xt[:, :], xT_in[:, t * P:(t + 1) * P])
            pm = psum.tile([P, TW], f32, tag="pm")
            nc.tensor.matmul(pm[:, :], lhsT=xt[:, :], rhs=wt_s[0][:, :],
                             start=True, stop=False)
            nc.tensor.matmul(pm[:, :], lhsT=ones1[:, :], rhs=b_s[0][0:1, :],
                             start=False, stop=True)
            transform_tail(t, pm, 0)

        def do_allgather(layer):
            nc.gpsimd.collective_compute(
                "AllGather", mybir.AluOpType.bypass, replica_groups=rg,
                ins=[xl_loc[layer][:, :].opt()],
                outs=[xl_full[layer][:, :].opt()])

        do_allgather(0)

        # ---- edge phases
        def edge_phase(layer, pc):
            s0 = 0          # slot-column offset into padded stream
            k0 = 0          # call offset
            for cols, tiles in cfg.tinfo:
                ncall = cols // CPC
                idx_c = gp.tile([P, ncall * 64], i16, tag="idx")
                nc.sync.dma_start(idx_c[:, :],
                                  idx_in[:, k0 * 64:(k0 + ncall) * 64])
                w3d = gp.tile([P, cols, 256], bf16, tag="w")
                for k in range(ncall):
                    nc.gpsimd.dma_gather(
                        w3d[:, k * CPC:(k + 1) * CPC, :],
                        xl_full[layer][:, :],
                        idx_c[:, k * 64:(k + 1) * 64],
                        KCALL, KCALL, 256, single_packet=True,
                        queue_num=(k0 + k) % 4)
                wv = w3d[:, :, :].rearrange("p c (q f) -> p (c q) f", q=4)
                mv = m01_s[:, 4 * s0:4 * (s0 + cols)].rearrange(
                    "p (c q) -> p c q", q=4)
                a = zp.tile([P, cols, H], bf16, tag="a")
                b = zp.tile([P, cols, H], bf16, tag="b")
                zb = zp.tile([P, cols, H], bf16, tag="zb")
                L = w3d[:, :, :].rearrange("p c (q f) -> p c q f", q=4)
                nc.vector.tensor_tensor(
                    a[:, :, :], L[:, :, 0, :],
                    mv[:, :, 0:1].to_broadcast([P, cols, H]), MULT)
                nc.vector.tensor_tensor(
                    b[:, :, :], L[:, :, 1, :],
                    mv[:, :, 1:2].to_broadcast([P, cols, H]), MULT)
                nc.vector.tensor_tensor(a[:, :, :], a[:, :, :], b[:, :, :],
                                        ADD)
                nc.vector.tensor_tensor(
                    b[:, :, :], L[:, :, 2, :],
                    mv[:, :, 2:3].to_broadcast([P, cols, H]), MULT)
                nc.vector.tensor_tensor(a[:, :, :], a[:, :, :], b[:, :, :],
                                        ADD)
                nc.vector.tensor_tensor(
                    b[:, :, :], L[:, :, 3, :],
                    mv[:, :, 3:4].to_broadcast([P, cols, H]), MULT)
                nc.vector.tensor_tensor(zb[:, :, :], a[:, :, :], b[:, :, :],
                                        ADD)
                for t, c0, d in tiles:
                    nc.vector.tensor_tensor(
                        zb[:, c0:c0 + d, :], zb[:, c0:c0 + d, :],
                        xr_res[layer][:, t * H:(t + 1) * H]
                        .unsqueeze(1).to_broadcast([P, d, H]), ADD)
                e = tp.tile([P, cols], f32, tag="e")
                rn = tp.tile([P, cols], f32, tag="rn")
                nc.vector.tensor_reduce(e[:, :], zb[:, :, 0:pc], X, ADD)
                nc.vector.tensor_reduce(rn[:, :], zb[:, :, pc:H], X, ADD)
                nc.vector.tensor_tensor(e[:, :], e[:, :], rn[:, :], SUB)
                nc.scalar.activation(a[:, :, :], zb[:, :, :], AF.Abs)
                ap_ = tp.tile([P, cols], f32, tag="ap")
                an_ = tp.tile([P, cols], f32, tag="an")
                nc.vector.tensor_reduce(ap_[:, :], a[:, :, 0:pc], X, ADD)
                nc.vector.tensor_reduce(an_[:, :], a[:, :, pc:H], X, ADD)
                nc.vector.tensor_tensor(ap_[:, :], ap_[:, :], an_[:, :], SUB)
                nc.vector.tensor_scalar(ap_[:, :], ap_[:, :], 2.0 / 3.0, None,
                                        MULT)
                nc.vector.tensor_tensor(e[:, :], e[:, :], ap_[:, :], ADD)
                nc.vector.tensor_tensor(
                    e[:, :], e[:, :], mneg_s[:, s0:s0 + cols], ADD)
                ex = tp.tile([P, cols], bf16, tag="ex")
                for t, c0, d in tiles:
                    nc.scalar.activation(
                        ex[:, c0:c0 + d], e[:, c0:c0 + d],
                        AF.Exp, scale=0.6,
                        accum_out=den_res[:, t:t + 1])
                nc.vector.tensor_tensor(
                    zb[:, :, :], zb[:, :, :],
                    ex[:, :].unsqueeze(2).to_broadcast([P, cols, H]), MULT)
                t0 = tiles[0][0]
                ntc = len(tiles)
                numer_c = tp.tile([P, ntc, H], f32, tag="numer")
                for i, (t, c0, d) in enumerate(tiles):
                    nc.vector.tensor_reduce(
                        numer_c[:, i, :],
                        zb[:, c0:c0 + d, :].transpose([0, 2, 1]),
                        X, ADD)
                den_c = tp.tile([P, ntc], f32, tag="denc")
                nc.vector.tensor_scalar(den_c[:, :], den_res[:, t0:t0 + ntc],
                                        1e-30, None, ADD)
                rden_c = tp.tile([P, ntc], f32, tag="rdenc")
                nc.vector.reciprocal(rden_c[:, :], den_c[:, :])
                den_b = tp.tile([P, ntc], bf16, tag="denb")
                nc.scalar.copy(den_b[:, :], den_res[:, t0:t0 + ntc])
                xr_v = xr_res[layer][:, t0 * H:(t0 + ntc) * H].rearrange(
                    "p (t h) -> p t h", h=H)
                t2_c = tp.tile([P, ntc, H], f32, tag="t2c")
                nc.vector.tensor_tensor(
                    t2_c[:, :, :], xr_v,
                    den_b[:, :].unsqueeze(2).to_broadcast([P, ntc, H]), MULT)
                nc.vector.tensor_tensor(numer_c[:, :, :], numer_c[:, :, :],
                                        t2_c[:, :, :], SUB)
                nc.vector.tensor_tensor(
                    numer_c[:, :, :], numer_c[:, :, :],
                    rden_c[:, :].unsqueeze(2).to_broadcast([P, ntc, H]), MULT)
                nc.vector.tensor_tensor(
                    numer_c[:, :, :], numer_c[:, :, :],
                    inva_s[layer][:, :].unsqueeze(1).to_broadcast([P, ntc, H]),
                    MULT)
                nc.vector.tensor_tensor(
                    numer_c[:, :, :], numer_c[:, :, :],
                    xd_res[layer][:, t0 * H:(t0 + ntc) * H].rearrange(
                        "p (t h) -> p t h", h=H), ADD)
                h_c = tp.tile([P, ntc, H], f32, tag="hc")
                nc.scalar.activation(h_c[:, :, :], numer_c[:, :, :], AF.Relu)
                if layer == 0:
                    for i, (t, c0, d) in enumerate(tiles):
                        pt = psum.tile([H, P], f32, tag="pt")
                        nc.tensor.transpose(pt[:, :], h_c[:, i, :], ident[:, :])
                        hT = tp.tile([H, P], f32, tag="hT")
                        nc.scalar.copy(hT[:, :], pt[:, :])
                        pm = psum.tile([P, TW], f32, tag="pm2")
                        nc.tensor.matmul(pm[:, :], lhsT=hT[:, :],
                                         rhs=wt_s[1][:, :],
                                         start=True, stop=False)
                        nc.tensor.matmul(pm[:, :], lhsT=ones1[:, :],
                                         rhs=b_s[1][0:1, :],
                                         start=False, stop=True)
                        transform_tail(t, pm, 1)
                else:
                    fo_c = tp.tile([P, ntc, H], f32, tag="foc")
                    nc.vector.tensor_tensor(
                        fo_c[:, :, :], h_c[:, :, :],
                        w3_s[:, :].unsqueeze(1).to_broadcast([P, ntc, H]),
                        MULT)
                    nc.vector.tensor_reduce(out3_s[:, t0:t0 + ntc],
                                            fo_c[:, :, :], X, ADD)
                s0 += cols
                k0 += ncall

        edge_phase(0, cfg.p1)
        do_allgather(1)
        edge_phase(1, cfg.p2)

        nc.vector.tensor_scalar(out3_s[:, :], out3_s[:, :], float(cfg.b3val),
                                None, ADD)
        nc.sync.dma_start(out3[:, :], out3_s[:, :])

    nc.finalize()
    return nc


# ------------------------------------------------------------------ kernel

def make_inputs_and_cfg(inputs, N, F_IN, NLOC):
    g = prep_graph(inputs["edge_index"], N, NLOC)
    w1 = prep_layer_weights(inputs["Wl1"], inputs["bl1"], inputs["Wr1"],
                            inputs["br1"], inputs["att1"], inputs["b1"],
                            inputs["Wd1"], inputs["bd1"])
    w2 = prep_layer_weights(inputs["Wl2"], inputs["bl2"], inputs["Wr2"],
                            inputs["br2"], inputs["att2"], inputs["b2"],
                            inputs["Wd2"], inputs["bd2"], in_perm=w1["perm"])
    x = np.asarray(inputs["x"], np.float32)
    W3p = np.asarray(inputs["W3"], np.float32)[w2["perm"]]
    b3val = float(np.asarray(inputs["b3"], np.float32)[0])
    cfg = Cfg(N, F_IN, NLOC, g["tinfo"], w1["p_cnt"], w2["p_cnt"], b3val,
              g["NCALL"], g["SDP"])

    w3rep = np.broadcast_to(W3p[:, 0][None, :], (P, H)).copy()
    inva1 = np.broadcast_to(w1["inva"][None, :], (P, H)).copy()
    inva2 = np.broadcast_to(w2["inva"][None, :], (P, H)).copy()
    sgn1 = np.broadcast_to(w1["sgn"][None, :], (P, H)).copy()
    sgn2 = np.broadcast_to(w2["sgn"][None, :], (P, H)).copy()
    b1rep = np.broadcast_to(w1["brow"][None, :], (P, 3 * H)).copy()
    b2rep = np.broadcast_to(w2["brow"][None, :], (P, 3 * H)).copy()

    in_maps = []
    order = g["order"]
    for c in range(NCORES):
        ii = np.arange(c, N, NCORES)
        lp = ii // NCORES
        x_loc = np.zeros((NLOC, F_IN), dtype=np.float32)
        x_loc[lp] = x[order[ii]]
        in_maps.append({
            "xT_loc": np.ascontiguousarray(x_loc.T),
            "idx_all": g["idx_all"][c],
            "mneg": g["mneg"][c],
            "m01": g["m01"][c],
            "wt1": w1["Wt"], "wt2": w2["Wt"],
            "b1rep": b1rep, "b2rep": b2rep,
            "inva1": inva1, "inva2": inva2, "w3rep": w3rep,
            "sgn1": sgn1, "sgn2": sgn2,
        })
    return cfg, in_maps, g


def unshard_output(results, g, N, NLOC):
    out = np.zeros((N, 1), dtype=np.float32)
    order = g["order"]
    for c in range(NCORES):
        o = np.asarray(results[c]["out3"])          # [128, NT]
        ii = np.arange(c, N, NCORES)
        lp = ii // NCORES
        out[order[ii], 0] = o[lp % P, lp // P]
    return out


def kernel(**inputs):
    from concourse.bass_utils import run_bass_kernel_spmd
    N, F_IN, NLOC = 100000, 128, 12544
    cfg, in_maps, g = make_inputs_and_cfg(inputs, N, F_IN, NLOC)
    nc = build_bass(cfg)
    res = run_bass_kernel_spmd(nc, in_maps, core_ids=list(range(NCORES)))
    return unshard_output(res.results, g, N, NLOC)


# revision 8
# speedup vs baseline: 1.0557x; 1.0557x over previous
"""Trainium2 Bass kernel v2 for the 2-layer GATv2 + dense-skip GNN.

Key change vs v1: the per-edge gather uses gpsimd.dma_gather (K=1024-index
single-packet calls, ~6ns/slot on the Q7) instead of per-slot
indirect_dma_start (~13.7ns/slot).  dma_gather requires int16 indices, so the
gather table is stored as 4-node *lines* of 256 bf16 (512 B): line id =
node>>2 (25088 < 32767 fits int16).  Each slot gathers its whole 4-node line;
the 3 wrong subrows (and pad slots) are killed in the softmax by adding -1e8
to their logits (host-precomputed mask), so no on-chip select is needed:
softmax weights of dead subrows are exactly 0 and the weighted aggregation
ignores them.

Other changes: no-max softmax (logits are O(1); exp is shift-invariant),
e = 0.6*(Rp-Rn) + 0.4*(Ap-An) computed from signed/abs reduces of the
|att|-scaled z (the 0.6 folds into the Exp's scale), aggregation runs on the
raw gathered xl (no xr correction needed), xr/xd stay SBUF-resident, x is
host-pre-transposed so layer-1 transforms skip the PE transpose, biases are
added from a replicated SBUF row via DVE instead of 1-row matmuls, and
layer-2 transforms are fused per-tile into the layer-1 edge phase.
"""
import sys
import numpy as np
import ml_dtypes

sys.path.insert(0, "/opt/trn_rl_repo")

P = 128
H = 64
NCORES = 8
KCALL = 1024          # dma_gather indices per call (single-packet max)
CPC = 8               # slot-columns per call (KCALL/P)
CH_COLS = 64          # slot-columns per slab chunk (multiple of CPC)
BIG = 1.0e8


class Cfg:
    def __init__(self, N, F_IN, NLOC, tinfo, p1, p2, b3val, ncall, sdp):
        self.N = N
        self.F_IN = F_IN
        self.NLOC = NLOC
        self.NT = NLOC // P
        self.NPAD = NCORES * NLOC
        self.tinfo = tinfo        # list of (tile, col0, d_t) per chunk
        self.p1 = p1
        self.p2 = p2
        self.b3val = b3val
        self.NCALL = ncall        # calls per layer
        self.SDP = sdp            # padded slot-columns per layer


# ---------------------------------------------------------------- host prep

def prep_graph(edge_index, N, NLOC):
    src = np.asarray(edge_index[0], dtype=np.int64)
    dst = np.asarray(edge_index[1], dtype=np.int64)
    NT = NLOC // P
    NPAD = NCORES * NLOC
    deg = np.bincount(dst, minlength=N)
    order = np.argsort(-deg, kind="stable")
    g_of_old = np.empty(N, dtype=np.int64)
    ii = np.arange(N)
    g_of_old[order] = (ii % NCORES) * NLOC + (ii // NCORES)
    deg_sorted = deg[order]
    d_t = [max(1, int(deg_sorted[NCORES * P * t])) if NCORES * P * t < N else 1
           for t in range(NT)]

    # chunks of tiles, slot-columns padded to CPC multiples per chunk
    chunks = []          # list of list of (tile, col0_in_chunk, d_t)
    cur, cols = [], 0
    for t in range(NT):
        if cols + d_t[t] > CH_COLS and cur:
            chunks.append(cur)
            cur, cols = [], 0
        cur.append((t, cols, d_t[t]))
        cols += d_t[t]
    if cur:
        chunks.append(cur)
    ch_cols = []
    for ch in chunks:
        c = sum(d for _, _, d in ch)
        ch_cols.append(-(-c // CPC) * CPC)
    SDP = sum(ch_cols)
    NCALL = SDP // CPC

    dst_g = g_of_old[dst]
    es = np.argsort(dst_g, kind="stable")
    src_g_sorted = g_of_old[src[es]].astype(np.int64)
    dst_g_sorted = dst_g[es]
    node_start = np.searchsorted(dst_g_sorted, np.arange(NPAD), side="left")
    node_end = np.searchsorted(dst_g_sorted, np.arange(NPAD), side="right")

    # per-core padded slot grids: line idx (int16), pad mask, subrow select
    lidx = np.zeros((NCORES, P, SDP), dtype=np.int16)
    mneg = np.full((NCORES, P, SDP), -BIG, dtype=np.float32)
    m01 = np.zeros((NCORES, P, SDP, 4), dtype=ml_dtypes.bfloat16)
    for c in range(NCORES):
        base = c * NLOC
        ns = node_start[base:base + NLOC]
        ne = node_end[base:base + NLOC]
        s0 = 0
        for ch in chunks:
            for t, c0, d in ch:
                for p in range(P):
                    lp = t * P + p
                    k = min(ne[lp] - ns[lp], d)
                    if k <= 0:
                        continue
                    srcs = src_g_sorted[ns[lp]:ns[lp] + k]
                    lidx[c, p, s0 + c0:s0 + c0 + k] = (srcs >> 2).astype(np.int16)
                    mneg[c, p, s0 + c0:s0 + c0 + k] = 0.0
                    m01[c, p, np.arange(s0 + c0, s0 + c0 + k), srcs & 3] = 1.0
            s0 += -(-sum(d for _, _, d in ch) // CPC) * CPC
    assert s0 == SDP

    # idx stream: [128, NCALL*64] int16, wrapped in 16 partitions, replicated
    idx_all = np.zeros((NCORES, P, NCALL * (KCALL // 16)), dtype=np.int16)
    for c in range(NCORES):
        for k in range(NCALL):
            blk = lidx[c, :, k * CPC:(k + 1) * CPC]       # [128, 8]
            fl = blk.T.reshape(-1)                        # pos i = s*128+p
            wrapped = fl.reshape(KCALL // 16, 16).T       # [16, 64]
            for g in range(8):
                idx_all[c, g * 16:(g + 1) * 16,
                        k * 64:(k + 1) * 64] = wrapped

    # chunk info: list over chunks of (padded_cols, [(tile, col0, d)])
    tinfo = []
    for ch, pc in zip(chunks, ch_cols):
        tinfo.append((pc, ch))
    return dict(order=order, tinfo=tinfo, SDP=SDP, NCALL=NCALL,
                idx_all=idx_all, mneg=mneg,
                m01=m01.reshape(NCORES, P, SDP * 4))


def prep_layer_weights(Wl, bl, Wr, br, att, b, Wd, bd, in_perm=None):
    Wl, bl, Wr, br, att, b, Wd, bd = [np.asarray(a, np.float64) for a in
                                      (Wl, bl, Wr, br, att, b, Wd, bd)]
    if in_perm is not None:
        Wl, Wr, Wd = Wl[in_perm], Wr[in_perm], Wd[in_perm]
    s = np.where(att >= 0, 1.0, -1.0)
    a = np.maximum(np.abs(att), 1e-12)
    perm = np.argsort(-s, kind="stable")
    p_cnt = int((s > 0).sum())
    ap = a[perm]
    # cols: [Wl*|att|]perm | [Wr*|att|]perm | Wd perm   (192 wide, no bias)
    Wt = np.concatenate([
        (Wl * a[None, :])[:, perm],
        (Wr * a[None, :])[:, perm],
        Wd[:, perm]], axis=1)
    brow = np.concatenate([
        np.zeros(H),
        ((bl + br) * a)[perm],
        (bd + b + bl)[perm]])
    inva = 1.0 / ap
    return dict(Wt=Wt.astype(np.float32),
                brow=brow.astype(np.float32),
                inva=inva.astype(np.float32), sgn=s[perm].astype(np.float32),
                perm=perm, p_cnt=p_cnt)


# ------------------------------------------------------------- bass builder

def build_bass(cfg):
    import concourse.bass as bass
    import concourse.bacc as bacc
    import concourse.mybir as mybir
    import concourse.tile as tile
    from concourse.masks import make_identity
    from contextlib import ExitStack

    f32 = mybir.dt.float32
    bf16 = mybir.dt.bfloat16
    i16 = mybir.dt.int16
    X = mybir.AxisListType.X
    ADD = mybir.AluOpType.add
    SUB = mybir.AluOpType.subtract
    MULT = mybir.AluOpType.mult
    AF = mybir.ActivationFunctionType

    NT, NLOC, NPAD, F_IN = cfg.NT, cfg.NLOC, cfg.NPAD, cfg.F_IN
    SDP, NCALL = cfg.SDP, cfg.NCALL
    TW = 3 * H                     # 192
    NLINES = NPAD // 4

    nc = bacc.Bacc("TRN2", target_bir_lowering=False, num_devices=NCORES,
                   num_swdge_queues=4)

    xT_in = nc.dram_tensor("xT_loc", [F_IN, NLOC], f32, kind="ExternalInput")
    idx_in = nc.dram_tensor("idx_all", [P, NCALL * 64], i16,
                            kind="ExternalInput")
    mneg_in = nc.dram_tensor("mneg", [P, SDP], f32, kind="ExternalInput")
    m01_in = nc.dram_tensor("m01", [P, SDP * 4], bf16, kind="ExternalInput")
    wt1_in = nc.dram_tensor("wt1", [F_IN, TW], f32, kind="ExternalInput")
    wt2_in = nc.dram_tensor("wt2", [H, TW], f32, kind="ExternalInput")
    b1_in = nc.dram_tensor("b1rep", [P, TW], f32, kind="ExternalInput")
    b2_in = nc.dram_tensor("b2rep", [P, TW], f32, kind="ExternalInput")
    inva1_in = nc.dram_tensor("inva1", [P, H], f32, kind="ExternalInput")
    inva2_in = nc.dram_tensor("inva2", [P, H], f32, kind="ExternalInput")
    sgn1_in = nc.dram_tensor("sgn1", [P, H], f32, kind="ExternalInput")
    sgn2_in = nc.dram_tensor("sgn2", [P, H], f32, kind="ExternalInput")
    w3_in = nc.dram_tensor("w3rep", [P, H], f32, kind="ExternalInput")
    out3 = nc.dram_tensor("out3", [P, NT], f32, kind="ExternalOutput")

    rg = [list(range(NCORES))]

    with ExitStack() as ctx:
        tc = ctx.enter_context(tile.TileContext(nc))
        dram = ctx.enter_context(tc.tile_pool(name="dram", bufs=1, space="DRAM"))
        xl_loc = [dram.tile([NLOC, H], bf16, name=f"xl{l}_loc") for l in (1, 2)]
        xl_full = [dram.tile([NLINES, 256], bf16, name=f"xl{l}_full",
                             addr_space="Shared") for l in (1, 2)]

        const = ctx.enter_context(tc.tile_pool(name="const", bufs=1))
        ident = const.tile([P, P], f32)
        make_identity(nc, ident[:, :])
        ones1 = const.tile([1, P], f32)
        nc.vector.memset(ones1[:, :], 1.0)
        wt_s, b_s, inva_s, sgn_s = [], [], [], []
        for l, (wt_i, b_i, iv_i, sg_i, kdim) in enumerate([
                (wt1_in, b1_in, inva1_in, sgn1_in, F_IN),
                (wt2_in, b2_in, inva2_in, sgn2_in, H)]):
            w = const.tile([kdim, TW], f32, name=f"wt{l}_s")
            nc.sync.dma_start(w[:, :], wt_i[:, :])
            b = const.tile([P, TW], f32, name=f"b{l}_s")
            nc.sync.dma_start(b[:, :], b_i[:, :])
            iv = const.tile([P, H], f32, name=f"iv{l}_s")
            nc.sync.dma_start(iv[:, :], iv_i[:, :])
            sg = const.tile([P, H], bf16, name=f"sg{l}_s")
            nc.gpsimd.dma_start(sg[:, :], sg_i[:, :])
            wt_s.append(w); b_s.append(b); inva_s.append(iv); sgn_s.append(sg)
        w3_s = const.tile([P, H], f32)
        nc.sync.dma_start(w3_s[:, :], w3_in[:, :])
        mneg_s = const.tile([P, SDP], f32)
        nc.sync.dma_start(mneg_s[:, :], mneg_in[:, :])
        m01_s = const.tile([P, SDP * 4], bf16)
        nc.sync.dma_start(m01_s[:, :], m01_in[:, :])
        # residents: xr/xd per layer (bf16), den per layer (f32)
        xr_res = [const.tile([P, NT * H], bf16, name=f"xr{l}") for l in (1, 2)]
        xd_res = [const.tile([P, NT * H], f32, name=f"xd{l}") for l in (1, 2)]
        den_res = const.tile([P, NT], f32)
        out3_s = const.tile([P, NT], f32)

        psum = ctx.enter_context(tc.tile_pool(name="psum", bufs=2, space="PSUM"))
        gp = ctx.enter_context(tc.tile_pool(name="gp", bufs=2))
        zp = ctx.enter_context(tc.tile_pool(name="zp", bufs=1))
        tp = ctx.enter_context(tc.tile_pool(name="tp", bufs=2))

        def transform_tail(t, pm, layer):
            """pm: psum [P, TW] for tile t of `layer` (0/1). Write residents +
            xl table rows (bf16) to DRAM."""
            r0 = t * H
            nc.scalar.copy(xr_res[layer][:, r0:r0 + H], pm[:, H:2 * H])
            nc.scalar.copy(xd_res[layer][:, r0:r0 + H], pm[:, 2 * H:TW])
            ot = tp.tile([P, H], bf16, tag="ot")
            nc.scalar.copy(ot[:, :], pm[:, 0:H])
            nc.sync.dma_start(xl_loc[layer][t * P:(t + 1) * P, :], ot[:, :])

        # ---- phase T1: layer-1 transforms from pre-transposed x
        for t in range(NT):
            xt = tp.tile([F_IN, P], f32, tag="xt")
            nc.sync.dma_start(xt[:, :], xT_in[:, t * P:(t + 1) * P])
            pm = psum.tile([P, TW], f32, tag="pm")
            nc.tensor.matmul(pm[:, :], lhsT=xt[:, :], rhs=wt_s[0][:, :],
                             start=True, stop=False)
            nc.tensor.matmul(pm[:, :], lhsT=ones1[:, :], rhs=b_s[0][0:1, :],
                             start=False, stop=True)
            transform_tail(t, pm, 0)

        def do_allgather(layer):
            nc.gpsimd.collective_compute(
                "AllGather", mybir.AluOpType.bypass, replica_groups=rg,
                ins=[xl_loc[layer][:, :].opt()],
                outs=[xl_full[layer][:, :].opt()])

        do_allgather(0)

        # ---- edge phases
        def edge_phase(layer, pc):
            s0 = 0          # slot-column offset into padded stream
            k0 = 0          # call offset
            for cols, tiles in cfg.tinfo:
                ncall = cols // CPC
                idx_c = gp.tile([P, ncall * 64], i16, tag="idx")
                nc.sync.dma_start(idx_c[:, :],
                                  idx_in[:, k0 * 64:(k0 + ncall) * 64])
                w3d = gp.tile([P, cols, 256], bf16, tag="w")
                for k in range(ncall):
                    nc.gpsimd.dma_gather(
                        w3d[:, k * CPC:(k + 1) * CPC, :],
                        xl_full[layer][:, :],
                        idx_c[:, k * 64:(k + 1) * 64],
                        KCALL, KCALL, 256, single_packet=True,
                        queue_num=(k0 + k) % 4)
                wv = w3d[:, :, :].rearrange("p c (q f) -> p (c q) f", q=4)
                mv = m01_s[:, 4 * s0:4 * (s0 + cols)].rearrange(
                    "p (c q) -> p c q", q=4)
                a = zp.tile([P, cols, H], bf16, tag="a")
                b = zp.tile([P, cols, H], bf16, tag="b")
                zb = zp.tile([P, cols, H], bf16, tag="zb")
                L = w3d[:, :, :].rearrange("p c (q f) -> p c q f", q=4)
                nc.vector.tensor_tensor(
                    a[:, :, :], L[:, :, 0, :],
                    mv[:, :, 0:1].to_broadcast([P, cols, H]), MULT)
                nc.vector.tensor_tensor(
                    b[:, :, :], L[:, :, 1, :],
                    mv[:, :, 1:2].to_broadcast([P, cols, H]), MULT)
                nc.vector.tensor_tensor(a[:, :, :], a[:, :, :], b[:, :, :],
                                        ADD)
                nc.vector.tensor_tensor(
                    b[:, :, :], L[:, :, 2, :],
                    mv[:, :, 2:3].to_broadcast([P, cols, H]), MULT)
                nc.vector.tensor_tensor(a[:, :, :], a[:, :, :], b[:, :, :],
                                        ADD)
                nc.vector.tensor_tensor(
                    b[:, :, :], L[:, :, 3, :],
                    mv[:, :, 3:4].to_broadcast([P, cols, H]), MULT)
                nc.vector.tensor_tensor(zb[:, :, :], a[:, :, :], b[:, :, :],
                                        ADD)
                for t, c0, d in tiles:
                    nc.vector.tensor_tensor(
                        zb[:, c0:c0 + d, :], zb[:, c0:c0 + d, :],
                        xr_res[layer][:, t * H:(t + 1) * H]
                        .unsqueeze(1).to_broadcast([P, d, H]), ADD)
                e = tp.tile([P, cols], f32, tag="e")
                rn = tp.tile([P, cols], f32, tag="rn")
                nc.vector.tensor_reduce(e[:, :], zb[:, :, 0:pc], X, ADD)
                nc.vector.tensor_reduce(rn[:, :], zb[:, :, pc:H], X, ADD)
                nc.vector.tensor_tensor(e[:, :], e[:, :], rn[:, :], SUB)
                nc.scalar.activation(a[:, :, :], zb[:, :, :], AF.Abs)
                ap_ = tp.tile([P, cols], f32, tag="ap")
                an_ = tp.tile([P, cols], f32, tag="an")
                nc.vector.tensor_reduce(ap_[:, :], a[:, :, 0:pc], X, ADD)
                nc.vector.tensor_reduce(an_[:, :], a[:, :, pc:H], X, ADD)
                nc.vector.tensor_tensor(ap_[:, :], ap_[:, :], an_[:, :], SUB)
                nc.vector.tensor_scalar(ap_[:, :], ap_[:, :], 2.0 / 3.0, None,
                                        MULT)
                nc.vector.tensor_tensor(e[:, :], e[:, :], ap_[:, :], ADD)
                nc.vector.tensor_tensor(
                    e[:, :], e[:, :], mneg_s[:, s0:s0 + cols], ADD)
                ex = tp.tile([P, cols], bf16, tag="ex")
                for t, c0, d in tiles:
                    nc.scalar.activation(
                        ex[:, c0:c0 + d], e[:, c0:c0 + d],
                        AF.Exp, scale=0.6,
                        accum_out=den_res[:, t:t + 1])
                nc.vector.tensor_tensor(
                    zb[:, :, :], zb[:, :, :],
                    ex[:, :].unsqueeze(2).to_broadcast([P, cols, H]), MULT)
                t0 = tiles[0][0]
                ntc = len(tiles)
                numer_c = tp.tile([P, ntc, H], f32, tag="numer")
                for i, (t, c0, d) in enumerate(tiles):
                    nc.vector.tensor_reduce(
                        numer_c[:, i, :],
                        zb[:, c0:c0 + d, :].transpose([0, 2, 1]),
                        X, ADD)
                den_c = tp.tile([P, ntc], f32, tag="denc")
                nc.vector.tensor_scalar(den_c[:, :], den_res[:, t0:t0 + ntc],
                                        1e-30, None, ADD)
                rden_c = tp.tile([P, ntc], f32, tag="rdenc")
                nc.vector.reciprocal(rden_c[:, :], den_c[:, :])
                den_b = tp.tile([P, ntc], bf16, tag="denb")
                nc.scalar.copy(den_b[:, :], den_res[:, t0:t0 + ntc])
                xr_v = xr_res[layer][:, t0 * H:(t0 + ntc) * H].rearrange(
                    "p (t h) -> p t h", h=H)
                t2_c = tp.tile([P, ntc, H], f32, tag="t2c")
                nc.vector.tensor_tensor(
                    t2_c[:, :, :], xr_v,
                    den_b[:, :].unsqueeze(2).to_broadcast([P, ntc, H]), MULT)
                nc.vector.tensor_tensor(numer_c[:, :, :], numer_c[:, :, :],
                                        t2_c[:, :, :], SUB)
                nc.vector.tensor_tensor(
                    numer_c[:, :, :], numer_c[:, :, :],
                    rden_c[:, :].unsqueeze(2).to_broadcast([P, ntc, H]), MULT)
                nc.vector.tensor_tensor(
                    numer_c[:, :, :], numer_c[:, :, :],
                    inva_s[layer][:, :].unsqueeze(1).to_broadcast([P, ntc, H]),
                    MULT)
                nc.vector.tensor_tensor(
                    numer_c[:, :, :], numer_c[:, :, :],
                    xd_res[layer][:, t0 * H:(t0 + ntc) * H].rearrange(
                        "p (t h) -> p t h", h=H), ADD)
                h_c = tp.tile([P, ntc, H], f32, tag="hc")
                nc.scalar.activation(h_c[:, :, :], numer_c[:, :, :], AF.Relu)
                if layer == 0:
                    for i, (t, c0, d) in enumerate(tiles):
                        pt = psum.tile([H, P], f32, tag="pt")
                        nc.tensor.transpose(pt[:, :], h_c[:, i, :], ident[:, :])
                        hT = tp.tile([H, P], f32, tag="hT")
                        nc.scalar.copy(hT[:, :], pt[:, :])
                        pm = psum.tile([P, TW], f32, tag="pm2")
                        nc.tensor.matmul(pm[:, :], lhsT=hT[:, :],
                                         rhs=wt_s[1][:, :],
                                         start=True, stop=False)
                        nc.tensor.matmul(pm[:, :], lhsT=ones1[:, :],
                                         rhs=b_s[1][0:1, :],
                                         start=False, stop=True)
                        transform_tail(t, pm, 1)
                else:
                    fo_c = tp.tile([P, ntc, H], f32, tag="foc")
                    nc.vector.tensor_tensor(
                        fo_c[:, :, :], h_c[:, :, :],
                        w3_s[:, :].unsqueeze(1).to_broadcast([P, ntc, H]),
                        MULT)
                    nc.vector.tensor_reduce(out3_s[:, t0:t0 + ntc],
                                            fo_c[:, :, :], X, ADD)
                s0 += cols
                k0 += ncall

        edge_phase(0, cfg.p1)
        do_allgather(1)
        edge_phase(1, cfg.p2)

        nc.vector.tensor_scalar(out3_s[:, :], out3_s[:, :], float(cfg.b3val),
                                None, ADD)
        nc.sync.dma_start(out3[:, :], out3_s[:, :])

    nc.finalize()
    return nc


# ------------------------------------------------------------------ kernel

def make_inputs_and_cfg(inputs, N, F_IN, NLOC):
    g = prep_graph(inputs["edge_index"], N, NLOC)
    w1 = prep_layer_weights(inputs["Wl1"], inputs["bl1"], inputs["Wr1"],
                            inputs["br1"], inputs["att1"], inputs["b1"],
                            inputs["Wd1"], inputs["bd1"])
    w2 = prep_layer_weights(inputs["Wl2"], inputs["bl2"], inputs["Wr2"],
                            inputs["br2"], inputs["att2"], inputs["b2"],
                            inputs["Wd2"], inputs["bd2"], in_perm=w1["perm"])
    x = np.asarray(inputs["x"], np.float32)
    W3p = np.asarray(inputs["W3"], np.float32)[w2["perm"]]
    b3val = float(np.asarray(inputs["b3"], np.float32)[0])
    cfg = Cfg(N, F_IN, NLOC, g["tinfo"], w1["p_cnt"], w2["p_cnt"], b3val,
              g["NCALL"], g["SDP"])

    w3rep = np.broadcast_to(W3p[:, 0][None, :], (P, H)).copy()
    inva1 = np.broadcast_to(w1["inva"][None, :], (P, H)).copy()
    inva2 = np.broadcast_to(w2["inva"][None, :], (P, H)).copy()
    sgn1 = np.broadcast_to(w1["sgn"][None, :], (P, H)).copy()
    sgn2 = np.broadcast_to(w2["sgn"][None, :], (P, H)).copy()
    b1rep = np.broadcast_to(w1["brow"][None, :], (P, 3 * H)).copy()
    b2rep = np.broadcast_to(w2["brow"][None, :], (P, 3 * H)).copy()

    in_maps = []
    order = g["order"]
    for c in range(NCORES):
        ii = np.arange(c, N, NCORES)
        lp = ii // NCORES
        x_loc = np.zeros((NLOC, F_IN), dtype=np.float32)
        x_loc[lp] = x[order[ii]]
        in_maps.append({
            "xT_loc": np.ascontiguousarray(x_loc.T),
            "idx_all": g["idx_all"][c],
            "mneg": g["mneg"][c],
            "m01": g["m01"][c],
            "wt1": w1["Wt"], "wt2": w2["Wt"],
            "b1rep": b1rep, "b2rep": b2rep,
            "inva1": inva1, "inva2": inva2, "w3rep": w3rep,
            "sgn1": sgn1, "sgn2": sgn2,
        })
    return cfg, in_maps, g


def unshard_output(results, g, N, NLOC):
    out = np.zeros((N, 1), dtype=np.float32)
    order = g["order"]
    for c in range(NCORES):
        o = np.asarray(results[c]["out3"])          # [128, NT]
        ii = np.arange(c, N, NCORES)
        lp = ii // NCORES
        out[order[ii], 0] = o[lp % P, lp // P]
    return out


def kernel(**inputs):
    from concourse.bass_utils import run_bass_kernel_spmd
    N, F_IN, NLOC = 100000, 128, 12544
    cfg, in_maps, g = make_inputs_and_cfg(inputs, N, F_IN, NLOC)
    nc = build_bass(cfg)
    res = run_bass_kernel_spmd(nc, in_maps, core_ids=list(range(NCORES)))
    return unshard_output(res.results, g, N, NLOC)
